# revision 1
# baseline (speedup 1.0000x reference)
"""Trainium2 Bass kernel for nn_GAT_Top (2-layer GAT + FC/BN + DistMult edge head).

Self-contained: takes FULL inputs, shards across 8 NeuronCores internally
(dst-node ownership for the sparse phases, node-parallel dense phases, halo
exchange via AllGather of node-feature tables), returns the FULL output.

v2: bf16 gather tables + bf16 sparse matmuls, per-tile S^T path for adst
(no dst-side gathers), non-transpose train gathers + PE transposes,
2 SWDGE queues.
"""
import os
import numpy as np

import concourse.bass as bass
import concourse.bacc as bacc
import concourse.tile as tile
from concourse import mybir
from concourse.bass_utils import run_bass_kernel_spmd

F32 = mybir.dt.float32
BF16 = mybir.dt.bfloat16
I16 = mybir.dt.int16
AO = mybir.AluOpType
AF = mybir.ActivationFunctionType

# problem constants (hardcoded per harness contract)
N, E, TE = 10000, 160000, 65536
NCORES = 8
NPC = N // NCORES            # 1250 nodes per core
NBLK = 10                    # dst blocks of 128 per core (last has 98)
TEC = TE // NCORES           # 8192 train edges per core
F1, F2 = 384, 256
H1 = 8
C1 = 48
ROW1 = 512                   # hx1 row bf16: h(384) | asrc(8) | pad
ROW2 = 384                   # hx2 row bf16: h2(256) | asrc2(1) | pad
NEG_SLOPE = 0.2
BN_EPS = 1e-5
PADLOC = 999.0
EC = 2048                    # train sub-chunk

last_exec_time_ns = None
_PROG_CACHE = {}


def _wrap_idx(idx):
    """int idx [n] (n%16==0) -> int16 [128, n//16]: i at [i%16, i//16], replicated
    across the 8 groups of 16 partitions (gpsimd cores)."""
    n = idx.shape[0]
    w = np.asarray(idx, np.int16).reshape(n // 16, 16).T  # [16, n//16]
    return np.tile(w, (8, 1))


def _pack_slabs(w, nslab):
    """[K, M] -> [128, nslab, M] with [p, s, m] = w[s*128+p, m]."""
    K, M = w.shape
    out = np.zeros((128, nslab, M), w.dtype)
    for s in range(nslab):
        k0, k1 = s * 128, min((s + 1) * 128, K)
        out[: k1 - k0, s, :] = w[k0:k1, :]
    return out


def _pack_col(v, nslab):
    """[K] -> [128, nslab] with [p, s] = v[s*128+p]."""
    K = v.shape[0]
    out = np.zeros((128, nslab), v.dtype)
    for s in range(nslab):
        k0, k1 = s * 128, min((s + 1) * 128, K)
        out[: k1 - k0, s] = v[k0:k1]
    return out


def _prepare(inputs):
    """Host-side preprocessing: edge partitioning/sorting/padding + weight packing."""
    ei = np.asarray(inputs["edge_index"]).astype(np.int64)
    loops = np.arange(N, dtype=np.int64)
    src = np.concatenate([ei[0], loops])
    dst = np.concatenate([ei[1], loops])

    per_core = []
    tpb_max = 1
    for c in range(NCORES):
        sel = (dst // NPC) == c
        s_c, d_c = src[sel], dst[sel]
        order = np.argsort(d_c, kind="stable")
        s_c, d_c = s_c[order], d_c[order]
        blocks = []
        base = c * NPC
        for b in range(NBLK):
            lo, hi = base + b * 128, min(base + (b + 1) * 128, base + NPC)
            m = (d_c >= lo) & (d_c < hi)
            bs, bd = s_c[m], d_c[m] - lo
            blocks.append((bs, bd))
            tpb_max = max(tpb_max, (len(bs) + 127) // 128)
        per_core.append(blocks)
    TPB = tpb_max

    # per-core index blobs
    cores = []
    tid = np.asarray(inputs["train_edge_id"]).astype(np.int64)
    pair_nodes = ei[:, tid]  # [2, TE]
    for c in range(NCORES):
        hx_idx = np.zeros((NBLK, TPB * 128), np.int64)
        dstloc = np.full((NBLK, TPB * 128), PADLOC, np.float32)
        for b, (bs, bd) in enumerate(per_core[c]):
            n = len(bs)
            hx_idx[b, :n] = bs
            dstloc[b, :n] = bd.astype(np.float32)
        hx_w = np.concatenate([_wrap_idx(hx_idx[b]) for b in range(NBLK)], axis=1)
        # dstloc column blob [128, NBLK*TPB]: [p, b*TPB+t] = dstloc[b, t*128+p]
        dl = dstloc.reshape(NBLK, TPB, 128).transpose(2, 0, 1).reshape(128, NBLK * TPB)
        dl = np.ascontiguousarray(dl)
        # padmask [128, NBLK]: 1.0 where dst row does not exist (beyond NPC)
        pm = np.zeros((128, NBLK), np.float32)
        for b in range(NBLK):
            nd = min(128, NPC - b * 128)
            if nd < 128:
                pm[nd:, b] = 1.0
        # combined train idx: per 2048-chunk: [a_chunk | b_chunk]
        ca = pair_nodes[0, c * TEC:(c + 1) * TEC]
        cb = pair_nodes[1, c * TEC:(c + 1) * TEC]
        segs = []
        for e0 in range(0, TEC, EC):
            segs.append(ca[e0:e0 + EC])
            segs.append(cb[e0:e0 + EC])
        c_idx = _wrap_idx(np.concatenate(segs))
        cores.append(dict(hx_idx=hx_w, dstloc=dl, padmask=pm, c_idx=c_idx))

    # weights (shared across cores)
    g = {k: np.asarray(v).astype(np.float32) for k, v in inputs.items()
         if k not in ("edge_index", "train_edge_id")}
    A1s = np.zeros((F1, H1), np.float32)
    A1d = np.zeros((F1, H1), np.float32)
    for h in range(H1):
        A1s[h * C1:(h + 1) * C1, h] = g["gat1_asrc"][h]
        A1d[h * C1:(h + 1) * C1, h] = g["gat1_adst"][h]
    ga1 = np.concatenate([g["gat1_w"] @ A1s, g["gat1_w"] @ A1d], axis=1)  # [384,16]
    ga2 = np.concatenate([g["gat2_w"] @ g["gat2_asrc"].T,
                          g["gat2_w"] @ g["gat2_adst"].T], axis=1)        # [256,2]

    shared = dict(
        w1=_pack_slabs(g["fc1_w"], 3), b1c=_pack_col(g["fc1_b"], 3),
        g1=_pack_slabs(g["gat1_w"], 3), ga1=_pack_slabs(ga1, 3),
        w5=_pack_slabs(g["fc5_w"], 3), b5c=_pack_col(g["fc5_b"], 2),
        g2=_pack_slabs(g["gat2_w"], 2), ga2=_pack_slabs(ga2, 2),
        w2f=_pack_slabs(g["fc2_w"], 2),
        b2rep=np.tile(g["fc2_b"][None, :], (128, 1)).astype(np.float32),
        w4=_pack_slabs(g["fc4_w"].astype(np.float32), 2),
        b4c=g["fc4_b"].reshape(7, 1).astype(np.float32),
        bn1g=_pack_col(g["bn1_g"], 3), bn1b=_pack_col(g["bn1_b"], 3),
        bn2g=_pack_col(g["bn2_g"], 2), bn2b=_pack_col(g["bn2_b"], 2),
        iota=np.tile(np.arange(128, dtype=np.float32), (128, 1)),
        eye=np.eye(128, dtype=np.float32),
        ones_col=np.ones((128, 1), np.float32),
    )

    x = np.asarray(inputs["x"]).astype(np.float32)
    for c in range(NCORES):
        xc = x[c * NPC:(c + 1) * NPC]              # [1250, 384]
        cores[c]["xT"] = _pack_slabs(np.ascontiguousarray(xc.T), 3)
    return dict(TPB=TPB, cores=cores, shared=shared)


def _build_program(TPB):
    # debug bisect: stop building after phase PH (1..6); 6 = full program
    PH = int(os.environ.get("BASS_GAT_PHASES", "6"))
    nc = bacc.Bacc("TRN2", target_bir_lowering=False, debug=False,
                   num_devices=NCORES, num_swdge_queues=2)

    def din(name, shape, dt=F32):
        return nc.dram_tensor(name, list(shape), dt, kind="ExternalInput").ap()

    D = dict(
        xT=din("xT", [128, 3, NPC]),
        hx_idx=din("hx_idx", [128, NBLK * TPB * 8], I16),
        dstloc=din("dstloc", [128, NBLK * TPB]),
        padmask=din("padmask", [128, NBLK]),
        c_idx=din("c_idx", [128, 2 * TEC // 16], I16),
        w1=din("w1", [128, 3, F1]), b1c=din("b1c", [128, 3]),
        g1=din("g1", [128, 3, F1]), ga1=din("ga1", [128, 3, 16]),
        w5=din("w5", [128, 3, F2]), b5c=din("b5c", [128, 2]),
        g2=din("g2", [128, 2, F2]), ga2=din("ga2", [128, 2, 2]),
        w2f=din("w2f", [128, 2, F2]), b2rep=din("b2rep", [128, F2]),
        w4=din("w4", [128, 2, 7]), b4c=din("b4c", [7, 1]),
        bn1g=din("bn1g", [128, 3]), bn1b=din("bn1b", [128, 3]),
        bn2g=din("bn2g", [128, 2]), bn2b=din("bn2b", [128, 2]),
        iota=din("iota", [128, 128]), eye=din("eye", [128, 128]),
        ones_col=din("ones_col", [128, 1]),
    )
    out_t = nc.dram_tensor("out_t", [7, TEC], F32, kind="ExternalOutput").ap()

    with tile.TileContext(nc) as tc:
        with tc.tile_pool(name="persist", bufs=1) as pp, \
             tc.tile_pool(name="dram", bufs=1, space="DRAM") as dd:
            # ---- persistent SBUF loads ----
            sb = {}
            for k, ap in D.items():
                t = pp.tile(list(ap.shape), ap.dtype, tag=f"in_{k}")
                nc.sync.dma_start(out=t[:], in_=ap)
                sb[k] = t
            # bf16 copies of constants used by bf16 ops
            w4b = pp.tile([128, 2, 7], BF16, tag="w4b")
            nc.vector.tensor_copy(out=w4b[:], in_=sb["w4"][:])
            iota_b = pp.tile([128, 128], BF16, tag="iota_b")
            nc.vector.tensor_copy(out=iota_b[:], in_=sb["iota"][:])
            eye_b = pp.tile([128, 128], BF16, tag="eye_b")
            nc.vector.tensor_copy(out=eye_b[:], in_=sb["eye"][:])
            dstloc_b = pp.tile([128, NBLK * TPB], BF16, tag="dstloc_b")
            nc.vector.tensor_copy(out=dstloc_b[:], in_=sb["dstloc"][:])
            ones_b = pp.tile([128, 1], BF16, tag="ones_b")
            nc.vector.tensor_copy(out=ones_b[:], in_=sb["ones_col"][:])

            # ---- DRAM bounces ----
            hx1_b = dd.tile([NPC, ROW1], BF16, tag="hx1b")
            hx1_f = dd.tile([N, ROW1], BF16, tag="hx1f", addr_space="Shared")
            hx2_b = dd.tile([NPC, ROW2], BF16, tag="hx2b")
            hx2_f = dd.tile([N, ROW2], BF16, tag="hx2f", addr_space="Shared")
            ho_b = dd.tile([NPC, F2], BF16, tag="hob")
            ho_f = dd.tile([N, F2], BF16, tag="hof", addr_space="Shared")
            bn1_i = dd.tile([1, 2 * F1], F32, tag="bn1i")
            bn1_o = dd.tile([1, 2 * F1], F32, tag="bn1o", addr_space="Shared")
            bn2_i = dd.tile([1, 2 * F2], F32, tag="bn2i")
            bn2_o = dd.tile([1, 2 * F2], F32, tag="bn2o", addr_space="Shared")

            # persistent activations
            z5T = pp.tile([128, 2, NPC], F32, tag="z5T")
            xgT = pp.tile([128, 3, NBLK * 128], F32, tag="xgT")
            xg2T = pp.tile([128, 2, NBLK * 128], F32, tag="xg2T")
            hmidT = pp.tile([128, 3, NPC], F32, tag="hmidT")
            hfinT = pp.tile([128, 2, NPC], F32, tag="hfinT")
            adst1o = pp.tile([128, NBLK, H1], BF16, tag="adst1o")
            adst2o = pp.tile([128, NBLK, 1], BF16, tag="adst2o")
            nc.vector.memset(adst1o[:], 0)
            nc.vector.memset(adst2o[:], 0)

            chunks = [(i, min(i + 512, NPC)) for i in range(0, NPC, 512)]

            # ================= Phase 1: dense1 (z1 -> h1/asrc/adst -> hx1)
            with tc.tile_pool(name="d1ps", bufs=1, space="PSUM") as d1ps, \
                 tc.tile_pool(name="d1sb", bufs=1) as d1sb:
                for (c0, c1) in chunks:
                    W = c1 - c0
                    zc = d1sb.tile([128, 3, 512], F32, tag="zc")
                    for m in range(3):
                        pz = d1ps.tile([128, 512], F32, tag="pz")
                        for k in range(3):
                            nc.tensor.matmul(
                                out=pz[:, :W],
                                lhsT=sb["w1"][:, k, m * 128:(m + 1) * 128],
                                rhs=sb["xT"][:, k, c0:c1],
                                start=(k == 0), stop=(k == 2))
                        nc.scalar.activation(out=zc[:, m, :W], in_=pz[:, :W],
                                             func=AF.Identity,
                                             bias=sb["b1c"][:, m:m + 1])
                    for nt0 in range(c0, c1, 128):
                        nt1 = min(nt0 + 128, c1)
                        R = nt1 - nt0
                        b = nt0 // 128
                        lo = nt0 - c0
                        ph = d1ps.tile([128, F1], F32, tag="ph")
                        pa = d1ps.tile([128, 16], F32, tag="pa")
                        for k in range(3):
                            nc.tensor.matmul(
                                out=ph[:R, :],
                                lhsT=zc[:, k, lo:lo + R],
                                rhs=sb["g1"][:, k, :],
                                start=(k == 0), stop=(k == 2))
                            nc.tensor.matmul(
                                out=pa[:R, :], lhsT=zc[:, k, lo:lo + R],
                                rhs=sb["ga1"][:, k, :],
                                start=(k == 0), stop=(k == 2))
                        hxt = d1sb.tile([128, ROW1], BF16, tag="hxt")
                        nc.scalar.activation(out=hxt[:R, 0:F1], in_=ph[:R, :],
                                             func=AF.Identity)
                        nc.vector.tensor_copy(out=hxt[:R, F1:F1 + H1],
                                              in_=pa[:R, 0:H1])
                        nc.vector.memset(hxt[:R, F1 + H1:ROW1], 0)
                        nc.sync.dma_start(out=hx1_b[nt0:nt1, :], in_=hxt[:R, :])
                        nc.vector.tensor_copy(out=adst1o[:R, b, :],
                                              in_=pa[:R, 8:16])

            rg = [list(range(NCORES))]

            def _early_out():
                nc.sync.dma_start(out=out_t[:, 0:128], in_=sb["iota"][0:7, 0:128])

            if PH >= 1:
                nc.gpsimd.collective_compute("AllGather", AO.bypass,
                                             ins=[hx1_b[:].opt()],
                                             outs=[hx1_f[:].opt()],
                                             replica_groups=rg)

            # ================= sparse GAT phase (shared builder)
            def sparse_phase(layer, table, xgTd, adsto, stats_sb):
                ROW = ROW1 if layer == 1 else ROW2
                F = F1 if layer == 1 else F2
                H = H1 if layer == 1 else 1
                NS = 3 if layer == 1 else 2
                nidx = TPB * 128
                with tc.tile_pool(name=f"sp{layer}", bufs=1, space="PSUM") as sp, \
                     tc.tile_pool(name=f"sl{layer}", bufs=1) as sl:
                    psum_sum = sp.tile([1, F], F32, tag="st0")
                    psum_ssq = sp.tile([1, F], F32, tag="st1")
                    for b in range(NBLK):
                        gth = sl.tile([128, TPB, ROW], BF16, tag="gth", bufs=2)
                        i0 = b * TPB * 8
                        nc.gpsimd.dma_gather(
                            gth[:], table[:], sb["hx_idx"][:, i0:i0 + TPB * 8],
                            num_idxs=nidx, num_idxs_reg=nidx, elem_size=ROW,
                            single_packet=False, queue_num=b % 2)
                        # pass 1: S tiles, S^T, adst columns
                        pda = sp.tile([128, (TPB + 1) * H], F32, tag="pda",
                                      bufs=2)
                        Ss = []
                        for t in range(TPB):
                            col = b * TPB + t
                            S = sl.tile([128, 128], BF16, tag=f"S{t}", bufs=2)
                            nc.vector.tensor_scalar(
                                out=S[:], in0=iota_b[:],
                                scalar1=sb["dstloc"][:, col:col + 1],
                                scalar2=None, op0=AO.is_equal)
                            Ss.append(S)
                            ptr = sp.tile([128, 128], BF16, tag="trSb", bufs=1)
                            nc.tensor.transpose(out=ptr[:], in_=S[:],
                                                identity=eye_b[:])
                            St = sl.tile([128, 128], BF16, tag="St", bufs=2)
                            nc.scalar.activation(out=St[:], in_=ptr[:],
                                                 func=AF.Identity)
                            nc.tensor.matmul(out=pda[:, t * H:(t + 1) * H],
                                             lhsT=St[:], rhs=adsto[:, b, :],
                                             start=True, stop=True)
                        # logits + w for the whole block
                        tl = sl.tile([128, TPB, H], F32, tag="tl", bufs=2)
                        tl2 = sl.tile([128, TPB, H], F32, tag="tl2", bufs=2)
                        wt = sl.tile([128, TPB, H], BF16 if H == H1 else F32,
                                      tag="wt", bufs=2)
                        nc.vector.tensor_tensor(
                            out=tl[:], in0=gth[:, :, F:F + H],
                            in1=pda[:, 0:TPB * H].rearrange(
                                "p (t h) -> p t h", h=H),
                            op=AO.add)
                        nc.vector.scalar_tensor_tensor(
                            out=tl2[:], in0=tl[:], scalar=NEG_SLOPE, in1=tl[:],
                            op0=AO.mult, op1=AO.max)
                        nc.scalar.activation(out=wt[:], in_=tl2[:], func=AF.Exp)
                        # pass 2: contrib + num/den matmuls
                        pnum = sp.tile([128, F], F32, tag="num", bufs=2)
                        for t in range(TPB):
                            col = b * TPB + t
                            if H == H1:
                                ct = sl.tile([128, F], BF16, tag="ct", bufs=2)
                                nc.vector.tensor_tensor(
                                    out=ct[:].rearrange("p (g c) -> p g c", c=C1),
                                    in0=gth[:, t, 0:F].rearrange(
                                        "p (g c) -> p g c", c=C1),
                                    in1=wt[:, t, :].to_broadcast([128, H, C1]),
                                    op=AO.mult)
                                lhs_num = Ss[t]
                                rhs_num = ct[:]
                                rhs_den = wt[:, t, :]
                            else:
                                S2 = sl.tile([128, 128], BF16, tag="S2", bufs=2)
                                nc.vector.tensor_scalar(
                                    out=S2[:], in0=iota_b[:],
                                    scalar1=sb["dstloc"][:, col:col + 1],
                                    scalar2=wt[:, t, 0:1], op0=AO.is_equal,
                                    op1=AO.mult)
                                lhs_num = S2
                                rhs_num = gth[:, t, 0:F]
                                rhs_den = ones_b[:]
                            nc.tensor.matmul(out=pnum[:], lhsT=lhs_num[:],
                                             rhs=rhs_num,
                                             start=(t == 0), stop=(t == TPB - 1))
                            nc.tensor.matmul(
                                out=pda[:, TPB * H:(TPB + 1) * H],
                                lhsT=Ss[t][:] if H == H1 else lhs_num[:],
                                rhs=rhs_den,
                                start=(t == 0), stop=(t == TPB - 1))
                        dent = sl.tile([128, H], F32, tag="dent", bufs=2)
                        nc.vector.tensor_scalar(
                            out=dent[:], in0=pda[:, TPB * H:(TPB + 1) * H],
                            scalar1=sb["padmask"][:, b:b + 1], scalar2=None,
                            op0=AO.add)
                        rec = sl.tile([128, H], F32, tag="rec", bufs=2)
                        nc.vector.reciprocal(out=rec[:], in_=dent[:])
                        xgt = sl.tile([128, F], F32, tag="xgt", bufs=2)
                        nc.vector.tensor_tensor(
                            out=xgt[:].rearrange("p (g c) -> p g c", g=H),
                            in0=pnum[:].rearrange("p (g c) -> p g c", g=H),
                            in1=rec[:].to_broadcast([128, H, F // H]),
                            op=AO.mult)
                        sq = sl.tile([128, F], F32, tag="sq", bufs=2)
                        nc.scalar.activation(out=sq[:], in_=xgt[:], func=AF.Square)
                        nc.tensor.matmul(out=psum_sum[:], lhsT=sb["ones_col"][:],
                                         rhs=xgt[:],
                                         start=(b == 0), stop=(b == NBLK - 1))
                        nc.tensor.matmul(out=psum_ssq[:], lhsT=sb["ones_col"][:],
                                         rhs=sq[:],
                                         start=(b == 0), stop=(b == NBLK - 1))
                        for s in range(NS):
                            ptx = sp.tile([128, 128], F32, tag="trS", bufs=1)
                            nc.tensor.transpose(out=ptx[:],
                                                in_=xgt[:, s * 128:(s + 1) * 128],
                                                identity=sb["eye"][:])
                            nc.scalar.activation(
                                out=xgTd[:, s, b * 128:(b + 1) * 128],
                                in_=ptx[:], func=AF.Identity)
                    nc.vector.tensor_copy(out=stats_sb[:, 0:F], in_=psum_sum[:])
                    nc.vector.tensor_copy(out=stats_sb[:, F:2 * F],
                                          in_=psum_ssq[:])

            if PH >= 2:
                stats1 = pp.tile([1, 2 * F1], F32, tag="stats1")
                sparse_phase(1, hx1_f, xgT, adst1o, stats1)
                nc.sync.dma_start(out=bn1_i[:], in_=stats1[:])
                nc.gpsimd.collective_compute("AllReduce", AO.add,
                                             ins=[bn1_i[:].opt()],
                                             outs=[bn1_o[:].opt()],
                                             replica_groups=rg)

            # ================= Phase: BN1 + residual + fc5 + h2/asrc2/adst2
            def bn_scale_shift(bn_o, F, NS, gcol, bcol, pool):
                gsum = pool.tile([128, NS], F32, tag="gsum")
                gssq = pool.tile([128, NS], F32, tag="gssq")
                nc.sync.dma_start(out=gsum[:], in_=bn_o[:, 0:F].rearrange(
                    "a (s p) -> (a p) s", p=128))
                nc.sync.dma_start(out=gssq[:], in_=bn_o[:, F:2 * F].rearrange(
                    "a (s p) -> (a p) s", p=128))
                mu = pool.tile([128, NS], F32, tag="mu")
                nc.vector.tensor_scalar(out=mu[:], in0=gsum[:], scalar1=1.0 / N,
                                        scalar2=None, op0=AO.mult)
                var = pool.tile([128, NS], F32, tag="var")
                nc.vector.tensor_scalar(out=var[:], in0=gssq[:], scalar1=1.0 / N,
                                        scalar2=None, op0=AO.mult)
                t1 = pool.tile([128, NS], F32, tag="t1")
                nc.vector.tensor_tensor(out=t1[:], in0=mu[:], in1=mu[:],
                                        op=AO.mult)
                nc.vector.tensor_tensor(out=var[:], in0=var[:], in1=t1[:],
                                        op=AO.subtract)
                nc.vector.tensor_scalar(out=var[:], in0=var[:], scalar1=BN_EPS,
                                        scalar2=None, op0=AO.add)
                sd = pool.tile([128, NS], F32, tag="sd")
                nc.scalar.activation(out=sd[:], in_=var[:], func=AF.Sqrt)
                rstd = pool.tile([128, NS], F32, tag="rstd")
                nc.vector.reciprocal(out=rstd[:], in_=sd[:])
                scale = pool.tile([128, NS], F32, tag="scale")
                nc.vector.tensor_tensor(out=scale[:], in0=gcol[:], in1=rstd[:],
                                        op=AO.mult)
                shift = pool.tile([128, NS], F32, tag="shift")
                nc.vector.tensor_tensor(out=shift[:], in0=mu[:], in1=scale[:],
                                        op=AO.mult)
                nc.vector.tensor_tensor(out=shift[:], in0=bcol[:], in1=shift[:],
                                        op=AO.subtract)
                return scale, shift

            if PH >= 3:
                with tc.tile_pool(name="bn1sb", bufs=1) as bnp, \
                     tc.tile_pool(name="d2ps", bufs=1, space="PSUM") as d2ps, \
                     tc.tile_pool(name="d2sb", bufs=1) as d2sb:
                    scale1, shift1 = bn_scale_shift(bn1_o, F1, 3, sb["bn1g"],
                                                    sb["bn1b"], bnp)
                    for s in range(3):
                        tmp = d2sb.tile([128, NPC], F32, tag="hmt", bufs=2)
                        nc.vector.scalar_tensor_tensor(
                            out=tmp[:], in0=xgT[:, s, 0:NPC],
                            scalar=scale1[:, s:s + 1], in1=sb["xT"][:, s, :],
                            op0=AO.mult, op1=AO.add)
                        nc.scalar.activation(out=hmidT[:, s, :], in_=tmp[:],
                                             func=AF.Relu, bias=shift1[:, s:s + 1])
                    # fc5 dense -> z5T
                    for (c0, c1) in chunks:
                        W = c1 - c0
                        for m in range(2):
                            pz = d2ps.tile([128, 512], F32, tag="pz5")
                            for k in range(3):
                                nc.tensor.matmul(
                                    out=pz[:, :W],
                                    lhsT=sb["w5"][:, k, m * 128:(m + 1) * 128],
                                    rhs=hmidT[:, k, c0:c1],
                                    start=(k == 0), stop=(k == 2))
                            nc.scalar.activation(out=z5T[:, m, c0:c1],
                                                 in_=pz[:, :W],
                                                 func=AF.Identity,
                                                 bias=sb["b5c"][:, m:m + 1])
                    # h2/asrc2/adst2 per node tile -> hx2
                    for nt0 in range(0, NPC, 128):
                        nt1 = min(nt0 + 128, NPC)
                        R = nt1 - nt0
                        b = nt0 // 128
                        ph = d2ps.tile([128, F2], F32, tag="ph2")
                        pa = d2ps.tile([128, 2], F32, tag="pa2")
                        for k in range(2):
                            nc.tensor.matmul(out=ph[:R, :],
                                             lhsT=z5T[:, k, nt0:nt1],
                                             rhs=sb["g2"][:, k, :],
                                             start=(k == 0), stop=(k == 1))
                            nc.tensor.matmul(out=pa[:R, :],
                                             lhsT=z5T[:, k, nt0:nt1],
                                             rhs=sb["ga2"][:, k, :],
                                             start=(k == 0), stop=(k == 1))
                        hxt = d2sb.tile([128, ROW2], BF16, tag="hxt2", bufs=2)
                        nc.scalar.activation(out=hxt[:R, 0:F2], in_=ph[:R, :],
                                             func=AF.Identity)
                        nc.vector.tensor_copy(out=hxt[:R, F2:F2 + 1],
                                              in_=pa[:R, 0:1])
                        nc.vector.memset(hxt[:R, F2 + 1:ROW2], 0)
                        nc.sync.dma_start(out=hx2_b[nt0:nt1, :], in_=hxt[:R, :])
                        nc.vector.tensor_copy(out=adst2o[:R, b, :],
                                              in_=pa[:R, 1:2])

                nc.gpsimd.collective_compute("AllGather", AO.bypass,
                                             ins=[hx2_b[:].opt()],
                                             outs=[hx2_f[:].opt()],
                                             replica_groups=rg)

            if PH >= 4:
                stats2 = pp.tile([1, 2 * F2], F32, tag="stats2")
                sparse_phase(2, hx2_f, xg2T, adst2o, stats2)
                nc.sync.dma_start(out=bn2_i[:], in_=stats2[:])
                nc.gpsimd.collective_compute("AllReduce", AO.add,
                                             ins=[bn2_i[:].opt()],
                                             outs=[bn2_o[:].opt()],
                                             replica_groups=rg)

            # ================= Phase: BN2 + residual + fc2 -> hout (bf16)
            if PH >= 5:
                with tc.tile_pool(name="bn2sb", bufs=1) as bnp2, \
                     tc.tile_pool(name="d3ps", bufs=1, space="PSUM") as d3ps, \
                     tc.tile_pool(name="d3sb", bufs=1) as d3sb:
                    scale2, shift2 = bn_scale_shift(bn2_o, F2, 2, sb["bn2g"],
                                                    sb["bn2b"], bnp2)
                    for s in range(2):
                        tmp = d3sb.tile([128, NPC], F32, tag="hft", bufs=2)
                        nc.vector.scalar_tensor_tensor(
                            out=tmp[:], in0=xg2T[:, s, 0:NPC],
                            scalar=scale2[:, s:s + 1], in1=z5T[:, s, :],
                            op0=AO.mult, op1=AO.add)
                        nc.scalar.activation(out=hfinT[:, s, :], in_=tmp[:],
                                             func=AF.Relu, bias=shift2[:, s:s + 1])
                    for nt0 in range(0, NPC, 128):
                        nt1 = min(nt0 + 128, NPC)
                        R = nt1 - nt0
                        po = d3ps.tile([128, F2], F32, tag="po")
                        for k in range(2):
                            nc.tensor.matmul(out=po[:R, :],
                                             lhsT=hfinT[:, k, nt0:nt1],
                                             rhs=sb["w2f"][:, k, :],
                                             start=(k == 0), stop=(k == 1))
                        hob = d3sb.tile([128, F2], BF16, tag="hob", bufs=2)
                        nc.vector.tensor_tensor(out=hob[:R, :], in0=po[:R, :],
                                                in1=sb["b2rep"][:R, :], op=AO.add)
                        nc.sync.dma_start(out=ho_b[nt0:nt1, :], in_=hob[:R, :])

                nc.gpsimd.collective_compute("AllGather", AO.bypass,
                                             ins=[ho_b[:].opt()],
                                             outs=[ho_f[:].opt()],
                                             replica_groups=rg)

            # ================= Phase: train-edge head
            if PH >= 6:
                with tc.tile_pool(name="tps", bufs=1, space="PSUM") as tps, \
                     tc.tile_pool(name="tsb", bufs=1) as tsb:
                    for ec in range(TEC // EC):
                        gab = tsb.tile([128, 2 * EC // 128, F2], BF16, tag="gab",
                                       bufs=2)
                        ic0 = ec * (2 * EC // 16)
                        nc.gpsimd.dma_gather(
                            gab[:], ho_f[:],
                            sb["c_idx"][:, ic0:ic0 + 2 * EC // 16],
                            num_idxs=2 * EC, num_idxs_reg=2 * EC, elem_size=F2,
                            single_packet=False, queue_num=ec % 2)
                        prn = tsb.tile([128, EC // 128, F2], BF16, tag="prn",
                                       bufs=2)
                        nc.vector.tensor_tensor(
                            out=prn[:], in0=gab[:, 0:EC // 128, :],
                            in1=gab[:, EC // 128:2 * EC // 128, :], op=AO.mult)
                        prT = tsb.tile([128, 2, EC], BF16, tag="prT", bufs=2)
                        for et in range(EC // 128):
                            for s in range(2):
                                ptx = tps.tile([128, 128], BF16, tag="ptt",
                                               bufs=2)
                                nc.tensor.transpose(
                                    out=ptx[:],
                                    in_=prn[:, et, s * 128:(s + 1) * 128],
                                    identity=eye_b[:])
                                nc.scalar.activation(
                                    out=prT[:, s, et * 128:(et + 1) * 128],
                                    in_=ptx[:], func=AF.Identity)
                        ot = tsb.tile([7, EC], F32, tag="ot", bufs=2)
                        for et in range(EC // 512):
                            po = tps.tile([7, 512], F32, tag="pot", bufs=2)
                            for k in range(2):
                                nc.tensor.matmul(
                                    out=po[:, :], lhsT=w4b[:, k, :],
                                    rhs=prT[:, k, et * 512:(et + 1) * 512],
                                    start=(k == 0), stop=(k == 1))
                            nc.scalar.activation(out=ot[:, et * 512:(et + 1) * 512],
                                                 in_=po[:, :], func=AF.Identity,
                                                 bias=sb["b4c"][:, 0:1])
                        nc.sync.dma_start(out=out_t[:, ec * EC:(ec + 1) * EC],
                                          in_=ot[:, :])
            if PH < 6:
                _early_out()

    nc.compile()
    return nc


def _make_in_maps(prep):
    shared = prep["shared"]
    maps = []
    for c in range(NCORES):
        m = {}
        for k in ("w1", "b1c", "g1", "ga1", "w5", "b5c", "g2", "ga2", "w2f",
                  "b2rep", "w4", "b4c", "bn1g", "bn1b", "bn2g", "bn2b",
                  "iota", "eye", "ones_col"):
            m[k] = np.ascontiguousarray(shared[k].astype(np.float32))
        cd = prep["cores"][c]
        m["xT"] = np.ascontiguousarray(cd["xT"].astype(np.float32))
        m["hx_idx"] = np.ascontiguousarray(cd["hx_idx"])
        m["dstloc"] = np.ascontiguousarray(cd["dstloc"])
        m["padmask"] = np.ascontiguousarray(cd["padmask"])
        m["c_idx"] = np.ascontiguousarray(cd["c_idx"])
        maps.append(m)
    return maps


def _ensure_ntff_hook():
    """Register the NTFF profile hook (missing antenv.axon_hooks shim)."""
    import sys, types
    if "antenv.axon_hooks" not in sys.modules:
        mod = types.ModuleType("antenv.axon_hooks")
        _h = [None]
        mod.set_axon_ntff_profile_hook = lambda h: _h.__setitem__(0, h)
        mod.get_axon_ntff_profile_hook = lambda: _h[0]
        sys.modules["antenv.axon_hooks"] = mod
        import antenv
        antenv.axon_hooks = mod
    import antenv.axon_hooks as ah
    if ah.get_axon_ntff_profile_hook() is None:
        try:
            from trn_agent_boot.trn_boot import _ntff_profile_via_ctypes
            ah.set_axon_ntff_profile_hook(
                _ntff_profile_via_ctypes("/opt/axon/libaxon_pjrt.so"))
        except Exception:
            pass


def kernel(**inputs):
    global last_exec_time_ns
    prep = _prepare(inputs)
    TPB = prep["TPB"]
    if TPB not in _PROG_CACHE:
        _PROG_CACHE[TPB] = _build_program(TPB)
    nc = _PROG_CACHE[TPB]
    in_maps = _make_in_maps(prep)
    trace = os.environ.get("BASS_GAT_TRACE", "0") == "1"
    if trace:
        _ensure_ntff_hook()
    res = run_bass_kernel_spmd(nc, in_maps, core_ids=list(range(NCORES)),
                               trace=trace)
    if trace:
        last_exec_time_ns = res.exec_time_ns
    out = np.concatenate(
        [res.results[c]["out_t"].T for c in range(NCORES)], axis=0)
    return out.astype(np.float32)

